# revision 5
# baseline (speedup 1.0000x reference)
"""Per-batch covariance on 8 Trainium2 NeuronCores.

Full input  : inputs [32, 8192, 128] f32
Full output : cov    [32, 128, 128] f32   (divide-by-N covariance)

Sharding: pure data parallel — batch dim split 4 per core, no collectives.

Per-core math for each batch item X [N=8192, D=128]:
    cov = (X^T X - colsum colsum^T / N) / N

X^T X accumulates on the PE over 64 [128,128] n-chunks (fp32 matmuls).
colsum is built without touching the PE's critical path: the DVE
pre-reduces each 1 MiB DMA tile over its 16 n-chunks ([128,16,128] ->
[128,128]), then one tiny PE matmul with constant `ones` weights folds
the partition axis into a row-form [1,128] PSUM accumulator. The mean
correction is a single K=1 rank-1 matmul accumulated into the same
PSUM bank as X^T X.

Constraint honored throughout: walrus allows only ONE semaphore wait
on a (fp32) Matmult, so every PE instruction is arranged to have at
most one cross-engine dependency (a warmup matmul absorbs the Pool
wait for the `ones` constant; PSUM pools are sized so no PE
instruction ever waits on a PSUM-slot release).
"""

import numpy as np

B, N, D = 32, 8192, 128
N_CORES = 8
B_PER = B // N_CORES  # 4 batch items per core

ROWS = 2048            # n-rows per DMA tile (1 MiB per tile)
TPB = ROWS // 128      # 16 [128,128] chunks per DMA tile
NBLK = N // ROWS       # 4 DMA tiles per batch item

_CACHE = {}


def _build_program():
    import concourse.bacc as bacc
    import concourse.mybir as mybir
    import concourse.tile as tile

    fp32 = mybir.dt.float32
    # Bacc (not raw Bass): its compile() pipeline splits multi-wait
    # instructions into event semaphores to satisfy the TRN2 1-wait-per-
    # instruction constraint walrus enforces.
    nc = bacc.Bacc(None)

    x = nc.declare_dram_parameter("inputs", [B_PER, N, D], fp32, isOutput=False)
    out = nc.declare_dram_parameter("cov", [B_PER, D, D], fp32, isOutput=True)

    with tile.TileContext(nc) as tc:
        with (
            tc.tile_pool(name="xin", bufs=3) as xin,
            tc.tile_pool(name="part", bufs=3) as part,
            tc.tile_pool(name="acc", bufs=B_PER, space="PSUM") as acc_pool,
            tc.tile_pool(name="rowp", bufs=B_PER, space="PSUM") as rowp_pool,
            tc.tile_pool(name="small", bufs=2 * B_PER) as small,
            tc.tile_pool(name="const", bufs=1) as const,
            tc.tile_pool(name="outp", bufs=2) as outp,
        ):
            ones = const.tile([128, 1], fp32)
            nc.gpsimd.memset(ones[:], 1.0)

            # Warmup matmul: absorbs the single Pool-sem wait for `ones`
            # so no later matmul ever needs it.
            warm = rowp_pool.tile([1, 1], fp32, tag="rowp")
            nc.tensor.matmul(warm[:], ones[:], ones[:])

            for b in range(B_PER):
                acc = acc_pool.tile([128, D], fp32)    # X^T X accumulator
                rowp = rowp_pool.tile([1, D], fp32, tag="rowp")  # colsum row
                for blk in range(NBLK):
                    xt = xin.tile([128, TPB, D], fp32)
                    src = x[b, blk * ROWS : (blk + 1) * ROWS, :].rearrange(
                        "(t p) d -> p t d", p=128
                    )
                    nc.sync.dma_start(xt[:], src)

                    # Partial n-sum of this tile on the DVE: [128,16,128]
                    # viewed as [p, d, t], reduce innermost (t).
                    pt = part.tile([128, D], fp32)
                    nc.vector.reduce_sum(
                        pt[:], xt[:].transpose([0, 2, 1]), axis=mybir.AxisListType.X
                    )

                    for t in range(TPB):
                        nc.tensor.matmul(
                            acc[:],
                            xt[:, t, :],
                            xt[:, t, :],
                            start=(blk == 0 and t == 0),
                            stop=False,
                        )
                    # Fold partition axis of pt: rowp += ones^T @ pt.
                    nc.tensor.matmul(
                        rowp[:],
                        ones[:],
                        pt[:],
                        start=(blk == 0),
                        stop=(blk == NBLK - 1),
                    )

                # Rank-1 mean correction: acc -= colsum colsum^T / N.
                c_row = small.tile([1, D], fp32)
                nc.scalar.copy(c_row[:], rowp[:])
                c_row_n = small.tile([1, D], fp32)
                nc.scalar.mul(c_row_n[:], rowp[:], -1.0 / N)
                nc.tensor.matmul(
                    acc[:],
                    c_row[:],
                    c_row_n[:],
                    start=False,
                    stop=True,
                )

                ot = outp.tile([128, D], fp32)
                nc.scalar.mul(ot[:], acc[:], 1.0 / N)
                nc.sync.dma_start(out[b], ot[:])

    nc.compile()
    return nc


def _get_program():
    if "nc" not in _CACHE:
        _CACHE["nc"] = _build_program()
    return _CACHE["nc"]


def kernel(**inputs) -> np.ndarray:
    from concourse.bass_utils import run_bass_kernel_spmd

    x = np.asarray(inputs["inputs"], dtype=np.float32)
    assert x.shape == (B, N, D), x.shape

    nc = _get_program()
    in_maps = [
        {"inputs": np.ascontiguousarray(x[c * B_PER : (c + 1) * B_PER])}
        for c in range(N_CORES)
    ]
    res = run_bass_kernel_spmd(nc, in_maps, list(range(N_CORES)))
    return np.concatenate([res.results[c]["cov"] for c in range(N_CORES)], axis=0)
